# revision 7
# baseline (speedup 1.0000x reference)
"""3-layer GAT (nn_GAT_56092272886249) on Trainium2, 8 NeuronCores, Bass/Tile.

Host plan: nodes relabeled by in-degree (desc) into 392 blocks of 128; block b
-> core b%8, position b//8. Per-position slot count uniform across cores ->
one SPMD program. Device tables T_l [NP, F+8] fp32 rows [feat | el | er] are
(re)built per layer by a dense bf16 matmul phase (redundant on all cores from
the AllGathered h). Edge phase per block: per-slot indirect row-gathers, masked
softmax natively on DVE (dst = partition), weighted accumulation, divide by
softmax sum, residual, ELU. Shards AllGathered (bf16) between layers.
"""
import sys
sys.path.insert(0, '/opt/trn_rl_repo')
import numpy as np
from contextlib import ExitStack

import concourse.bass as bass
import concourse.bacc as bacc
import concourse.tile as tile
import concourse.mybir as mybir
from concourse import bass_utils
from concourse.masks import make_identity

N, E, IN, H, D1, D3 = 50000, 800000, 256, 4, 64, 128
NCORES, P = 8, 128
F12, F3 = H * D1, H * D3          # 256, 512
TW12, TW3 = F12 + 8, F3 + 8      # 264, 520
SB = 8                            # slot sub-batch

_cache = {}


def _plan(src, dst):
    indeg = np.bincount(dst, minlength=N)
    order = np.argsort(-indeg, kind="stable")           # rank -> old id
    NB = ((N + P - 1) // P + NCORES - 1) // NCORES * NCORES  # 392
    NBC = NB // NCORES                                   # 49
    NP_ = NB * P                                         # 50176
    old2rank = np.empty(N, np.int64)
    old2rank[order] = np.arange(N)
    deg_r = np.zeros(NP_, np.int64)
    deg_r[:N] = indeg[order]
    # rank r -> (block, p) -> (core k, pos j) -> allgather row
    r = np.arange(NP_)
    blk, p_of = r // P, r % P
    k_of, j_of = blk % NCORES, blk // NCORES
    ag_of_rank = (k_of * NBC + j_of) * P + p_of
    D_pos = np.maximum(1, deg_r.reshape(NB, P).max(axis=1)
                       .reshape(NBC, NCORES).max(axis=1))   # [NBC]
    col0 = np.concatenate([[0], np.cumsum(D_pos)[:-1]]).astype(np.int64)
    SLOTD = int(D_pos.sum())
    SLOT = SLOTD + NBC                                    # + er self columns

    ag_of_old = ag_of_rank[old2rank]                      # old id -> AG row
    dst_r = old2rank[dst]                                 # edge dst as rank
    eorder = np.argsort(dst_r, kind="stable")
    ds, ss_ag = dst_r[eorder], ag_of_old[src[eorder]]
    starts = np.searchsorted(ds, np.arange(NP_))
    ends = np.searchsorted(ds, np.arange(NP_) + 1)

    idx = np.zeros((NCORES, P, SLOT), np.int32)
    msk = np.zeros((NCORES, P, SLOTD), np.float32)
    for rank in range(N):
        s0, s1 = starts[rank], ends[rank]
        if s0 == s1:
            continue
        b, pp = rank // P, rank % P
        k, j = b % NCORES, b // NCORES
        c = col0[j]
        idx[k, pp, c:c + (s1 - s0)] = ss_ag[s0:s1]
        msk[k, pp, c:c + (s1 - s0)] = 1.0
    for j in range(NBC):
        for k in range(NCORES):
            idx[k, :, SLOTD + j] = (k * NBC + j) * P + np.arange(P)
    return dict(order=order, old2rank=old2rank, ag_of_old=ag_of_old,
                ag_of_rank=ag_of_rank, NBC=NBC, NP=NP_, D_pos=D_pos,
                col0=col0, SLOTD=SLOTD, SLOT=SLOT, idx=idx, msk=msk)


def _build(plan):
    NBC, NP_, SLOT, SLOTD = plan["NBC"], plan["NP"], plan["SLOT"], plan["SLOTD"]
    D_pos, col0 = plan["D_pos"], plan["col0"]
    NT = NP_ // P
    dt = mybir.dt

    nc = bacc.Bacc("TRN2", target_bir_lowering=False, debug=False,
                   enable_asserts=False, num_devices=NCORES)
    t_xT = nc.dram_tensor("xT", [IN, NP_], dt.float32, kind="ExternalInput").ap()
    t_w1 = nc.dram_tensor("w1", [IN, TW12], dt.float32, kind="ExternalInput").ap()
    t_w2 = nc.dram_tensor("w2", [F12, TW12], dt.float32, kind="ExternalInput").ap()
    t_w3 = nc.dram_tensor("w3", [F12, TW3], dt.float32, kind="ExternalInput").ap()
    t_wr = nc.dram_tensor("wr", [F12, D3], dt.float32, kind="ExternalInput").ap()
    t_idx = nc.dram_tensor("idx", [P, SLOT], dt.int32, kind="ExternalInput").ap()
    t_msk = nc.dram_tensor("msk", [P, SLOTD], dt.float32, kind="ExternalInput").ap()
    o_out = nc.dram_tensor("out", [NBC * P, D3], dt.float32, kind="ExternalOutput").ap()

    with tile.TileContext(nc) as tc:
        with ExitStack() as ctx:
            const = ctx.enter_context(tc.tile_pool(name="const", bufs=1))
            sb = ctx.enter_context(tc.tile_pool(name="sb", bufs=3))
            gp = ctx.enter_context(tc.tile_pool(name="gp", bufs=3))
            acp = ctx.enter_context(tc.tile_pool(name="acp", bufs=2))
            ps = ctx.enter_context(tc.tile_pool(name="ps", bufs=2, space="PSUM"))
            dram = ctx.enter_context(tc.tile_pool(name="dram", bufs=1, space="DRAM"))

            T1 = dram.tile([NP_, TW12], dt.float32)
            T2 = dram.tile([NP_, TW12], dt.float32)
            T3 = dram.tile([NP_, TW3], dt.float32)
            h2d = dram.tile([NBC * P, F12], dt.float32)
            hfull = dram.tile([NP_, F12], dt.float32)
            resd = dram.tile([NBC * P, D3], dt.float32)

            ident = const.tile([P, P], dt.float32)
            make_identity(nc, ident[:])
            idx_t = const.tile([P, SLOT], dt.int32)
            nc.sync.dma_start(out=idx_t[:], in_=t_idx[:])
            msk_t = const.tile([P, SLOTD], dt.float32)
            nc.sync.dma_start(out=msk_t[:], in_=t_msk[:])
            def wload(t_ap, width, nm):
                parts = []
                for kk in range(2):
                    wt = const.tile([P, width], dt.float32, tag=f"{nm}{kk}")
                    nc.sync.dma_start(out=wt[:], in_=t_ap[kk * P:(kk + 1) * P, :])
                    parts.append(wt)
                return parts
            w1_t = wload(t_w1, TW12, "w1")
            w2_t = wload(t_w2, TW12, "w2")
            w3_t = wload(t_w3, TW3, "w3")
            wr_t = wload(t_wr, D3, "wr")

            def dense(use_x, w_tile, T_out, width):
                for t in range(NT):
                    lts = []
                    for kk in range(2):
                        lt = sb.tile([P, P], dt.float32, tag=f"lhsT{kk}")
                        if use_x:
                            nc.sync.dma_start(
                                out=lt[:],
                                in_=t_xT[kk * P:(kk + 1) * P, t * P:(t + 1) * P])
                        else:
                            ht = sb.tile([P, P], dt.float32, tag=f"hload{kk}")
                            nc.sync.dma_start(
                                out=ht[:],
                                in_=hfull[t * P:(t + 1) * P, kk * P:(kk + 1) * P])
                            tp = ps.tile([P, P], dt.float32, tag="tps")
                            nc.tensor.transpose(tp[:], ht[:], ident[:])
                            nc.vector.tensor_copy(out=lt[:], in_=tp[:])
                        lts.append(lt)
                    st = sb.tile([P, width], dt.float32, tag="dsb")
                    for gi, g0 in enumerate(range(0, width, 512)):
                        g1 = min(g0 + 512, width)
                        pt = ps.tile([P, g1 - g0], dt.float32, tag=f"dps{gi}")
                        for kk in range(2):
                            nc.tensor.matmul(pt[:], lhsT=lts[kk][:],
                                             rhs=w_tile[kk][:, g0:g1],
                                             start=(kk == 0), stop=(kk == 1))
                        nc.vector.tensor_copy(out=st[:, g0:g1], in_=pt[:])
                    nc.sync.dma_start(out=T_out[t * P:(t + 1) * P, :], in_=st[:])

            def edge(T, width, FD, out_cb):
                DH = FD // H
                for j in range(NBC):
                    Dj, c0 = int(D_pos[j]), int(col0[j])
                    er = gp.tile([P, H], dt.float32, tag="er")
                    nc.gpsimd.indirect_dma_start(
                        out=er[:], out_offset=None, in_=T[:],
                        in_offset=bass.IndirectOffsetOnAxis(
                            ap=idx_t[:, SLOTD + j:SLOTD + j + 1], axis=0),
                        element_offset=width - 4)
                    ssum = acp.tile([P, H], dt.float32, tag="ssum")
                    acc = acp.tile([P, FD], dt.float32, tag="acc")
                    for s0 in range(0, Dj, SB):
                        nb = min(SB, Dj - s0)
                        g = gp.tile([P, SB * width], dt.float32, tag="g")
                        for s in range(nb):
                            nc.gpsimd.indirect_dma_start(
                                out=g[:, s * width:(s + 1) * width],
                                out_offset=None, in_=T[:],
                                in_offset=bass.IndirectOffsetOnAxis(
                                    ap=idx_t[:, c0 + s0 + s:c0 + s0 + s + 1], axis=0))
                        gv = g[:, 0:nb * width].rearrange("p (s w) -> p s w", s=nb)
                        lg = gp.tile([P, SB * H], dt.float32, tag="lg")
                        lgv = lg[:, 0:nb * H].rearrange("p (s h) -> p s h", s=nb)
                        nc.vector.tensor_tensor(
                            out=lgv, in0=gv[:, :, FD:FD + 4],
                            in1=er[:, None, :].to_broadcast([P, nb, H]),
                            op=mybir.AluOpType.add)
                        lgn = gp.tile([P, SB * H], dt.float32, tag="lgn")
                        nc.vector.tensor_scalar(lgn[:, 0:nb * H], lg[:, 0:nb * H],
                                                0.0, 0.2,
                                                op0=mybir.AluOpType.min,
                                                op1=mybir.AluOpType.mult)
                        nc.vector.tensor_scalar_max(lg[:, 0:nb * H],
                                                    lg[:, 0:nb * H], 0.0)
                        nc.vector.tensor_tensor(out=lg[:, 0:nb * H],
                                                in0=lg[:, 0:nb * H],
                                                in1=lgn[:, 0:nb * H],
                                                op=mybir.AluOpType.add)
                        nc.scalar.activation(out=lg[:, 0:nb * H], in_=lg[:, 0:nb * H],
                                             func=mybir.ActivationFunctionType.Exp)
                        nc.vector.tensor_tensor(
                            out=lgv, in0=lgv,
                            in1=msk_t[:, c0 + s0:c0 + s0 + nb, None]
                                .to_broadcast([P, nb, H]),
                            op=mybir.AluOpType.mult)
                        pssum = gp.tile([P, H], dt.float32, tag="pssum")
                        nc.vector.reduce_sum(
                            out=pssum[:],
                            in_=lg[:, 0:nb * H].rearrange("p (s h) -> p h s", s=nb),
                            axis=mybir.AxisListType.X)
                        if s0 == 0:
                            nc.vector.tensor_copy(out=ssum[:], in_=pssum[:])
                        else:
                            nc.vector.tensor_tensor(out=ssum[:], in0=ssum[:],
                                                    in1=pssum[:],
                                                    op=mybir.AluOpType.add)
                        for s in range(nb):
                            wv = lgv[:, s, :, None].to_broadcast([P, H, DH])
                            fv = gv[:, s, 0:FD].rearrange("p (h d) -> p h d", h=H)
                            if s0 == 0 and s == 0:
                                nc.vector.tensor_tensor(
                                    out=acc[:].rearrange("p (h d) -> p h d", h=H),
                                    in0=fv, in1=wv, op=mybir.AluOpType.mult)
                            else:
                                tmp = gp.tile([P, FD], dt.float32, tag="tmp")
                                nc.vector.tensor_tensor(
                                    out=tmp[:].rearrange("p (h d) -> p h d", h=H),
                                    in0=fv, in1=wv, op=mybir.AluOpType.mult)
                                nc.vector.tensor_tensor(
                                    out=acc[:], in0=acc[:], in1=tmp[:],
                                    op=mybir.AluOpType.add)
                    nc.vector.tensor_scalar_max(ssum[:], ssum[:], 1e-9)
                    sinv = acp.tile([P, H], dt.float32, tag="sinv")
                    nc.vector.reciprocal(sinv[:], ssum[:])
                    nc.vector.tensor_tensor(
                        out=acc[:].rearrange("p (h d) -> p h d", h=H),
                        in0=acc[:].rearrange("p (h d) -> p h d", h=H),
                        in1=sinv[:, :, None].to_broadcast([P, H, DH]),
                        op=mybir.AluOpType.mult)
                    out_cb(j, acc)

            def elu_inplace(t, width):
                tm = sb.tile([P, width], dt.float32, tag="elutmp")
                nc.vector.tensor_scalar_min(tm[:], t[:], 0.0)
                nc.scalar.activation(out=tm[:], in_=tm[:],
                                     func=mybir.ActivationFunctionType.Exp)
                nc.vector.tensor_scalar_add(tm[:], tm[:], -1.0)
                nc.vector.tensor_scalar_max(t[:], t[:], 0.0)
                nc.vector.tensor_tensor(out=t[:], in0=t[:], in1=tm[:],
                                        op=mybir.AluOpType.add)

            def store_h(j, acc):
                nc.sync.dma_start(out=h2d[j * P:(j + 1) * P, :], in_=acc[:])

            def ag():
                nc.gpsimd.collective_compute(
                    "AllGather", mybir.AluOpType.bypass,
                    replica_groups=[list(range(NCORES))],
                    ins=[h2d.opt()], outs=[hfull.opt()])

            # ---------------- layer 1 ----------------
            dense(True, w1_t, T1[:], TW12)

            def l1_out(j, acc):
                elu_inplace(acc, F12)
                store_h(j, acc)
            edge(T1[:], TW12, F12, l1_out)
            ag()

            # ---------------- layer 2 ----------------
            dense(False, w2_t, T2[:], TW12)

            def l2_out(j, acc):
                hb = sb.tile([P, F12], dt.float32, tag="hres")
                nc.sync.dma_start(out=hb[:], in_=h2d[j * P:(j + 1) * P, :])
                nc.vector.tensor_tensor(out=acc[:], in0=acc[:], in1=hb[:],
                                        op=mybir.AluOpType.add)
                elu_inplace(acc, F12)
                store_h(j, acc)
            edge(T2[:], TW12, F12, l2_out)
            ag()

            # ---------------- layer 3 ----------------
            dense(False, w3_t, T3[:], TW3)
            for j in range(NBC):
                pt = ps.tile([P, D3], dt.float32, tag="rps")
                for kk in range(2):
                    ht = sb.tile([P, P], dt.float32, tag=f"hload{kk}")
                    nc.sync.dma_start(out=ht[:],
                                      in_=h2d[j * P:(j + 1) * P, kk * P:(kk + 1) * P])
                    tp = ps.tile([P, P], dt.float32, tag="tps")
                    nc.tensor.transpose(tp[:], ht[:], ident[:])
                    lt = sb.tile([P, P], dt.float32, tag=f"lhsT{kk}")
                    nc.vector.tensor_copy(out=lt[:], in_=tp[:])
                    nc.tensor.matmul(pt[:], lhsT=lt[:],
                                     rhs=wr_t[kk][:],
                                     start=(kk == 0), stop=(kk == 1))
                st = sb.tile([P, D3], dt.float32, tag="rsb")
                nc.vector.tensor_copy(out=st[:], in_=pt[:])
                nc.sync.dma_start(out=resd[j * P:(j + 1) * P, :], in_=st[:])

            def l3_out(j, acc):
                mn = sb.tile([P, D3], dt.float32, tag="mn")
                nc.vector.tensor_tensor(out=mn[:], in0=acc[:, 0:D3],
                                        in1=acc[:, D3:2 * D3],
                                        op=mybir.AluOpType.add)
                nc.vector.tensor_tensor(out=mn[:], in0=mn[:],
                                        in1=acc[:, 2 * D3:3 * D3],
                                        op=mybir.AluOpType.add)
                nc.vector.tensor_tensor(out=mn[:], in0=mn[:],
                                        in1=acc[:, 3 * D3:4 * D3],
                                        op=mybir.AluOpType.add)
                rs = sb.tile([P, D3], dt.float32, tag="rs")
                nc.sync.dma_start(out=rs[:], in_=resd[j * P:(j + 1) * P, :])
                nc.vector.tensor_scalar(mn[:], mn[:], 0.25, None,
                                        op0=mybir.AluOpType.mult)
                nc.vector.tensor_tensor(out=mn[:], in0=mn[:], in1=rs[:],
                                        op=mybir.AluOpType.add)
                nc.sync.dma_start(out=o_out[j * P:(j + 1) * P, :], in_=mn[:])
            edge(T3[:], TW3, F3, l3_out)

    nc.compile()
    return nc


def _prep_inputs(plan, x, W1, a1l, a1r, W2, a2l, a2r, W3, a3l, a3r, Wres3):
    NP_, ag_of_old = plan["NP"], plan["ag_of_old"]
    xT = np.zeros((IN, NP_), np.float32)
    xT[:, ag_of_old] = x.T
    def rhs(W, al, ar):
        alp = np.stack([W[:, h * (W.shape[1] // H):(h + 1) * (W.shape[1] // H)]
                        @ al[h] for h in range(H)], axis=1)
        arp = np.stack([W[:, h * (W.shape[1] // H):(h + 1) * (W.shape[1] // H)]
                        @ ar[h] for h in range(H)], axis=1)
        return np.concatenate([W, alp, arp], axis=1)
    return dict(
        xT=xT,
        w1=rhs(W1, a1l, a1r).astype(np.float32),
        w2=rhs(W2, a2l, a2r).astype(np.float32),
        w3=rhs(W3, a3l, a3r).astype(np.float32),
        wr=Wres3.reshape(F12, H, D3).mean(axis=1).astype(np.float32))


def kernel(x, src, dst, W1, a1l, a1r, b1, W2, a2l, a2r, b2, W3, a3l, a3r, b3,
           Wres3):
    assert not b1.any() and not b2.any(), "nonzero b1/b2 unsupported"
    if "plan" not in _cache:
        _cache["plan"] = _plan(np.asarray(src, np.int64), np.asarray(dst, np.int64))
    plan = _cache["plan"]
    if "nc" not in _cache:
        _cache["nc"] = _build(plan)
    nc = _cache["nc"]

    shared = _prep_inputs(plan, np.asarray(x, np.float32),
                          np.asarray(W1, np.float32), np.asarray(a1l, np.float32),
                          np.asarray(a1r, np.float32), np.asarray(W2, np.float32),
                          np.asarray(a2l, np.float32), np.asarray(a2r, np.float32),
                          np.asarray(W3, np.float32), np.asarray(a3l, np.float32),
                          np.asarray(a3r, np.float32), np.asarray(Wres3, np.float32))
    in_maps = [dict(shared, idx=plan["idx"][k], msk=plan["msk"][k])
               for k in range(NCORES)]
    res = bass_utils.run_bass_kernel_spmd(nc, in_maps, core_ids=list(range(NCORES)))

    NBC = plan["NBC"]
    out_full = np.empty((N, D3), np.float32)
    # output row of rank r lives on core k at row j*128+p
    rr = np.arange(N)
    blk, pp = rr // P, rr % P
    kk_, jj_ = blk % NCORES, blk // NCORES
    shards = np.stack([res.results[k]["out"] for k in range(NCORES)])  # [8, NBC*P, D3]
    out_full[plan["order"][rr]] = shards[kk_, jj_ * P + pp]
    out_full += np.asarray(b3, np.float32).reshape(H, D3).mean(axis=0)
    return out_full
